# revision 16
# baseline (speedup 1.0000x reference)
"""Trainium2 Bass kernel for nn_CostLearning quadratic cost:

    cost[i] = sum_d exp(q_diag_log[d]) * states[i,d]^2
            + sum_d exp(r_diag_log[d]) * actions[i,d]^2

Sharding: pure data parallel over B*T rows across 8 NeuronCores.
Per core: rows are laid out so SBUF partition p owns 256 *consecutive*
rows of the core's shard -> every DMA is 128 partitions x large
contiguous runs, and the d-reduction is a free-axis (X) segmented
reduce on the vector engine.

v2 layout (from trace analysis of v1 @ 68.1us):
  exec window = main_start .. last_teardown_event. v1 breakdown:
  2.2us first-DMA latency + 50.2us stream (SDMA ~100% busy @ ~400GB/s)
  + 5.3us compute drain + 1.8us store + 8.6us sem-teardown ladder.
  v2 attacks the non-stream parts:
  - tapered chunks: big (64 rows/partition, 4MB) early for low
    instruction count, tiny (4 rows) at the end so the post-stream
    drain is ~1.5us instead of 7us
  - squares written as bf16 (inputs stay f32): DVE segmented reduce
    runs at 2x, so the last chunk's reduce is short; f32 accumulate
    in the reduce keeps error ~1e-3 << 2e-2 gate
  - output stored in 5 pieces, 4 of them mid-stream; only a 16-row
    (8KB) store remains after the last input chunk
  - fewer total instructions -> shorter end-of-kernel semaphore
    teardown ladder
"""

import numpy as np

B, T, DS, DA = 128, 2048, 128, 32
BT = B * T
NCORES = 8
RPC = BT // NCORES        # rows per core = 32768
P = 128                   # SBUF partitions
NPP = RPC // P            # rows per partition = 256

# states chunks: uniform 1MB (16 rows/partition) for a fine-grained
# DMA/ACT/DVE pipeline, tapered tail (8,4,4) so the post-stream
# square+reduce drain is tiny. Lumpy 4MB chunks measurably stall the
# tail (v3: big reduces serialize ahead of the small tail chunks).
S_SCHED = [16] * 15 + [8, 4, 4]
assert sum(S_SCHED) == NPP

_cache = {}


def _build(weighted: bool):
    import concourse.bacc as bacc
    import concourse.bass as bass
    import concourse.tile as tile
    from concourse import mybir

    f32 = mybir.dt.float32
    bf16 = mybir.dt.bfloat16
    # all-bf16 intermediates: DVE's 2x_1P perf mode (2 elem/cycle)
    # requires every src AND dst of tensor_reduce to be 2-byte. The
    # reduce ALU accumulates in f32 internally; only the final write
    # rounds to bf16, so the error is ~2^-9 per cost term, well under
    # the 2e-2 gate.
    sq_dt = f32 if weighted else bf16
    red_dt = f32 if weighted else bf16
    nc = bacc.Bacc("TRN2", target_bir_lowering=False, debug=False)

    states = nc.dram_tensor("states", [RPC, DS], f32, kind="ExternalInput")
    actions = nc.dram_tensor("actions", [RPC, DA], f32, kind="ExternalInput")
    if weighted:
        qlog = nc.dram_tensor("qlog", [DS], f32, kind="ExternalInput")
        rlog = nc.dram_tensor("rlog", [DA], f32, kind="ExternalInput")
    cost = nc.dram_tensor("cost", [RPC], f32, kind="ExternalOutput")

    # partition p owns shard rows [p*NPP, (p+1)*NPP)
    sview = states[:].rearrange("(p n) d -> p n d", p=P)    # [128, 256, 128]
    aview = actions[:].rearrange("(p n) d -> p n d", p=P)   # [128, 256, 32]
    oview = cost[:].rearrange("(p n) -> p n", p=P)          # [128, 256]

    s_max = max(S_SCHED)
    a_max = 64

    with tile.TileContext(nc) as tc:
        with (
            tc.tile_pool(name="sio", bufs=6) as sio,
            tc.tile_pool(name="ssqp", bufs=4) as ssqp,
            tc.tile_pool(name="aio", bufs=3) as aio,
            tc.tile_pool(name="asqp", bufs=3) as asqp,
            tc.tile_pool(name="accp", bufs=1) as accp,
        ):
            st_red = accp.tile([P, NPP], red_dt)
            ac_red = accp.tile([P, NPP], red_dt)
            out_t = accp.tile([P, NPP], f32)

            if weighted:
                # exp(weights), broadcast to all partitions and tiled
                # along the free axis to match one chunk's [P, n, d]
                qrep = accp.tile([P, s_max, DS], f32)
                rrep = accp.tile([P, a_max, DA], f32)
                qap = qlog[:]
                rap = rlog[:]
                qb = bass.AP(tensor=qap.tensor, offset=qap.offset,
                             ap=[[0, P], [0, s_max], [1, DS]])
                rb = bass.AP(tensor=rap.tensor, offset=rap.offset,
                             ap=[[0, P], [0, a_max], [1, DA]])
                nc.gpsimd.dma_start(out=qrep, in_=qb)
                nc.gpsimd.dma_start(out=rrep, in_=rb)
                nc.scalar.activation(qrep, qrep,
                                     mybir.ActivationFunctionType.Exp)
                nc.scalar.activation(rrep, rrep,
                                     mybir.ActivationFunctionType.Exp)

            def do_schunk(row0, n):
                s_t = sio.tile([P, s_max, DS], f32, name="s_t")
                nc.sync.dma_start(out=s_t[:, :n, :],
                                  in_=sview[:, row0:row0 + n, :])
                ssq = ssqp.tile([P, s_max, DS], sq_dt, name="ssq")
                nc.scalar.activation(ssq[:, :n, :], s_t[:, :n, :],
                                     mybir.ActivationFunctionType.Square)
                if weighted:
                    nc.vector.tensor_mul(ssq[:, :n, :], ssq[:, :n, :],
                                         qrep[:, :n, :])
                with nc.allow_low_precision("bf16 cost partials; gate is 2e-2"):
                    nc.vector.reduce_sum(
                        out=st_red[:, row0:row0 + n],
                        in_=ssq[:, :n, :],
                        axis=mybir.AxisListType.X,
                    )

            def do_achunk(row0, n):
                a_t = aio.tile([P, a_max, DA], f32, name="a_t")
                nc.sync.dma_start(out=a_t[:, :n, :],
                                  in_=aview[:, row0:row0 + n, :])
                asq = asqp.tile([P, a_max, DA], sq_dt, name="asq")
                nc.scalar.activation(asq[:, :n, :], a_t[:, :n, :],
                                     mybir.ActivationFunctionType.Square)
                if weighted:
                    nc.vector.tensor_mul(asq[:, :n, :], asq[:, :n, :],
                                         rrep[:, :n, :])
                with nc.allow_low_precision("bf16 cost partials; gate is 2e-2"):
                    nc.vector.reduce_sum(
                        out=ac_red[:, row0:row0 + n],
                        in_=asq[:, :n, :],
                        axis=mybir.AxisListType.X,
                    )

            def finalize(r0, r1, store0=None, last=False):
                # add this region; store is a (row0, row1) range that may
                # cover several finalized regions. Mid-stream stores go on
                # the idle gpsimd (SWDGE) queue: HWDGE rings drain FIFO
                # per issuing engine, so a compute-gated store on the sync
                # ring would stall every later input DMA behind it. The
                # final store uses sync (lower latency; ring is empty by
                # then).
                nc.vector.tensor_add(out_t[:, r0:r1], st_red[:, r0:r1],
                                     ac_red[:, r0:r1])
                if store0 is not None:
                    eng = nc.sync if last else nc.gpsimd
                    eng.dma_start(out=oview[:, store0:r1],
                                  in_=out_t[:, store0:r1])

            # explicit interleaved emission: 1MB states chunks drive the
            # stream; action chunks and finalize/stores slot in as their
            # row ranges complete.
            do_schunk(0, 16); do_schunk(16, 16)
            do_achunk(0, 64)
            do_schunk(32, 16); do_schunk(48, 16)
            do_schunk(64, 16); do_schunk(80, 16)
            finalize(0, 64, store0=0)
            do_achunk(64, 64)
            do_schunk(96, 16); do_schunk(112, 16)
            do_schunk(128, 16); do_schunk(144, 16)
            finalize(64, 128, store0=64)
            do_achunk(128, 64)
            do_schunk(160, 16); do_schunk(176, 16)
            do_schunk(192, 16); do_schunk(208, 16)
            finalize(128, 192, store0=128)
            do_achunk(192, 32)
            do_schunk(224, 16)
            do_achunk(224, 16)
            finalize(192, 240, store0=192)
            do_schunk(240, 8)
            do_achunk(240, 16)
            do_schunk(248, 4)
            do_schunk(252, 4)
            finalize(240, 256, store0=240, last=True)

    nc.compile()
    return nc


# (kind, row0, nrows) in DMA issue order; states drive the stream
DMA_ORDER = [
    ('s', 0, 16), ('s', 16, 16), ('a', 0, 64), ('s', 32, 16), ('s', 48, 16),
    ('s', 64, 16), ('s', 80, 16), ('a', 64, 64), ('s', 96, 16), ('s', 112, 16),
    ('s', 128, 16), ('s', 144, 16), ('a', 128, 64), ('s', 160, 16), ('s', 176, 16),
    ('s', 192, 16), ('s', 208, 16), ('a', 192, 32), ('s', 224, 8), ('s', 232, 8),
    ('a', 224, 16), ('s', 240, 4), ('s', 244, 4), ('a', 240, 16), ('s', 248, 4),
    ('s', 252, 4),
]
assert sum(n for k, _, n in DMA_ORDER if k == 's') == NPP
assert sum(n for k, _, n in DMA_ORDER if k == 'a') == NPP

# finalize regions (row0, row1, a_idx, s_idxs) — derived below
REGIONS = [(0, 64), (64, 128), (128, 192), (192, 224), (224, 240), (240, 256)]

S_SLOTS, SQ_SLOTS = 6, 4
A_SLOTS, ASQ_SLOTS, FB_SLOTS = 3, 2, 2
S_MAX, A_MAX = 16, 64


def _build_manual():
    import concourse.bacc as bacc
    import concourse.bass as bass
    from concourse import mybir

    f32 = mybir.dt.float32
    nc = bacc.Bacc("TRN2", target_bir_lowering=False, debug=False)

    states = nc.dram_tensor("states", [RPC, DS], f32, kind="ExternalInput")
    actions = nc.dram_tensor("actions", [RPC, DA], f32, kind="ExternalInput")
    cost = nc.dram_tensor("cost", [RPC], f32, kind="ExternalOutput")

    sview = states[:].rearrange("(p n) d -> p n d", p=P)    # [128, 256, 128]
    aview = actions[:].rearrange("(p n) d -> p n d", p=P)   # [128, 256, 32]
    oview = cost[:].rearrange("(p n) -> p n", p=P)          # [128, 256]

    # --- static schedule bookkeeping ---------------------------------
    s_chunks = [(r, n) for k, r, n in DMA_ORDER if k == 's']
    a_chunks = [(r, n) for k, r, n in DMA_ORDER if k == 'a']
    # global square order = DMA order; map chunk -> global square index
    gsq = {}
    for g, (k, r, n) in enumerate(DMA_ORDER):
        gsq[(k, r)] = g
    # per-kind dma index
    sdma_idx = {r: i for i, (r, n) in enumerate(s_chunks)}
    adma_idx = {r: i for i, (r, n) in enumerate(a_chunks)}
    # DVE order: per region, s-reduces then a-reduce then add
    region_s = {i: [] for i in range(len(REGIONS))}
    region_a = {}
    for i, (r0, r1) in enumerate(REGIONS):
        for (r, n) in s_chunks:
            if r0 <= r < r1:
                region_s[i].append((r, n))
        for (r, n) in a_chunks:
            if r0 <= r < r1 or (r < r0 and r + n >= r1):
                region_a.setdefault(i, (r, n))
    # a-chunk [192:224] serves region 3; [224:240] region 4; [240:256] region 5
    # a-chunk [0:64] region 0 etc.  (each region has exactly one a chunk
    # covering it given the region boundaries above)
    # s-reduce global DVE completion order for ssq slot reuse:
    dve_s_order = []
    for i in range(len(REGIONS)):
        dve_s_order.extend(region_s[i])
    s_red_pos = {r: i for i, (r, n) in enumerate(dve_s_order)}

    from contextlib import ExitStack
    f32_ = f32
    es = ExitStack()
    with es:
        s_t = es.enter_context(nc.sbuf_tensor("s_t", [P, S_SLOTS, S_MAX, DS], f32))
        ssq = es.enter_context(nc.sbuf_tensor("ssq", [P, SQ_SLOTS, S_MAX, DS], f32))
        a_t = es.enter_context(nc.sbuf_tensor("a_t", [P, A_SLOTS, A_MAX, DA], f32))
        asq = es.enter_context(nc.sbuf_tensor("asq", [P, ASQ_SLOTS, A_MAX, DA], f32))
        fb1 = es.enter_context(nc.sbuf_tensor("fb1", [P, FB_SLOTS, A_MAX, 16], f32))
        fb2 = es.enter_context(nc.sbuf_tensor("fb2", [P, FB_SLOTS, A_MAX, 8], f32))
        st_red = es.enter_context(nc.sbuf_tensor("st_red", [P, NPP], f32))
        ac_red = es.enter_context(nc.sbuf_tensor("ac_red", [P, NPP], f32))
        out_t = es.enter_context(nc.sbuf_tensor("out_t", [P, NPP], f32))
        zbias = es.enter_context(nc.sbuf_tensor("zbias", [P, 1], f32))
        SEM_Z = es.enter_context(nc.semaphore("SEM_Z"))
        # lane count must EXCEED the max DMAs concurrently in flight
        # (S_SLOTS / A_SLOTS): two same-lane DMAs overlapping in flight
        # can alias their +16 completion incs when SDMA engines skew.
        SD = [es.enter_context(nc.semaphore(f"SEM_SD{j}")) for j in range(8)]
        AD = [es.enter_context(nc.semaphore(f"SEM_AD{j}")) for j in range(3)]
        SEM_SQ = es.enter_context(nc.semaphore("SEM_SQ"))
        SEM_F = es.enter_context(nc.semaphore("SEM_F"))
        SEM_F1 = es.enter_context(nc.semaphore("SEM_F1"))
        SEM_RED = es.enter_context(nc.semaphore("SEM_RED"))
        SEM_ARED = es.enter_context(nc.semaphore("SEM_ARED"))
        SEM_ADD = es.enter_context(nc.semaphore("SEM_ADD"))
        SEM_ST = es.enter_context(nc.semaphore("SEM_ST"))
        all_sems = [SEM_Z, *SD, *AD, SEM_SQ, SEM_F, SEM_F1, SEM_RED,
                    SEM_ARED, SEM_ADD, SEM_ST]
        sem_nums = sorted(s.num for s in all_sems)
        assert sem_nums == list(range(sem_nums[0], sem_nums[-1] + 1))
        sem_range = range(sem_nums[0], sem_nums[-1] + 1)
        block = es.enter_context(nc.Block())

        @block.sync
        def _(sync):
            n_s = n_a = 0
            for k, r, n in DMA_ORDER:
                if k == 's':
                    i = sdma_idx[r]
                    if i >= S_SLOTS:
                        # slot reuse: ACT must have squared chunk i-S_SLOTS
                        pr, _ = s_chunks[i - S_SLOTS]
                        sync.wait_ge(SEM_SQ, gsq[('s', pr)] + 1)
                    sl = i % S_SLOTS
                    sync.dma_start(
                        out=s_t[:, sl, :n, :], in_=sview[:, r:r + n, :]
                    ).then_inc(SD[i % 8], 16)
                    n_s += 1
                else:
                    i = adma_idx[r]
                    if i >= A_SLOTS:
                        pr, _ = a_chunks[i - A_SLOTS]
                        sync.wait_ge(SEM_SQ, gsq[('a', pr)] + 1)
                    sl = i % A_SLOTS
                    sync.dma_start(
                        out=a_t[:, sl, :n, :], in_=aview[:, r:r + n, :]
                    ).then_inc(AD[i % 3], 16)
                    n_a += 1
            # final store [192:256] after the last region add
            sync.wait_ge(SEM_ADD, len(REGIONS))
            sync.dma_start(out=oview[:, 192:], in_=out_t[:, 192:]).then_inc(
                SEM_ST, 16)

        @block.scalar
        def _(scalar):
            scalar.wait_ge(SEM_Z, 1)
            for g, (k, r, n) in enumerate(DMA_ORDER):
                if k == 's':
                    i = sdma_idx[r]
                    sl_in = i % S_SLOTS
                    sl_out = i % SQ_SLOTS
                    scalar.wait_ge(SD[i % 8], 16 * (i // 8 + 1))
                    if i >= SQ_SLOTS:
                        # ssq slot reuse: DVE reduce of chunk i-SQ_SLOTS done
                        pr, _ = s_chunks[i - SQ_SLOTS]
                        scalar.wait_ge(SEM_RED, s_red_pos[pr] + 1)
                    scalar.activation(
                        ssq[:, sl_out, :n, :], s_t[:, sl_in, :n, :],
                        mybir.ActivationFunctionType.Square,
                        bias=zbias[:, :1],
                    ).then_inc(SEM_SQ, 1)
                else:
                    i = adma_idx[r]
                    sl_in = i % A_SLOTS
                    sl_out = i % ASQ_SLOTS
                    scalar.wait_ge(AD[i % 3], 16 * (i // 3 + 1))
                    if i >= ASQ_SLOTS:
                        # asq slot reuse: gpsimd folds of chunk i-2 done
                        scalar.wait_ge(SEM_F, i - ASQ_SLOTS + 1)
                    scalar.activation(
                        asq[:, sl_out, :n, :], a_t[:, sl_in, :n, :],
                        mybir.ActivationFunctionType.Square,
                        bias=zbias[:, :1],
                    ).then_inc(SEM_SQ, 1)

        @block.gpsimd
        def _(gpsimd):
            # raw bass has no const pool; ACT Square needs a bias AP
            gpsimd.memset(zbias[:, :], 0.0).then_inc(SEM_Z, 1)
            for i, (r, n) in enumerate(a_chunks):
                sl = i % ASQ_SLOTS
                fsl = i % FB_SLOTS
                gpsimd.wait_ge(SEM_SQ, gsq[('a', r)] + 1)
                gpsimd.tensor_add(fb1[:, fsl, :n, :],
                                  asq[:, sl, :n, 0:16], asq[:, sl, :n, 16:32]
                                  ).then_inc(SEM_F1, 1)
                if i >= FB_SLOTS:
                    # fb2 slot reuse: DVE a-reduce of chunk i-2 done
                    gpsimd.wait_ge(SEM_ARED, i - FB_SLOTS + 1)
                gpsimd.wait_ge(SEM_F1, i + 1)
                gpsimd.tensor_add(fb2[:, fsl, :n, :],
                                  fb1[:, fsl, :n, 0:8], fb1[:, fsl, :n, 8:16]
                                  ).then_inc(SEM_F, 1)
                # mid-stream stores for regions 0..2 ride the SWDGE queue
                if i in (1, 2, 3):
                    reg = i - 1
                    gpsimd.wait_ge(SEM_ADD, reg + 1)
                    r0, r1 = REGIONS[reg]
                    gpsimd.dma_start(out=oview[:, r0:r1],
                                     in_=out_t[:, r0:r1]).then_inc(SEM_ST, 16)
            # kernel completion + reset for re-execution: every sem's final
            # inc is upstream of the 4 store completions
            gpsimd.wait_ge(SEM_ST, 64)
            gpsimd.dma_reset(sem_range)
            gpsimd.sem_clear(sem_range)

        @block.vector
        def _(vector):
            adds_done = 0
            cum_s = 0
            for ri, (r0, r1) in enumerate(REGIONS):
                for (r, n) in region_s[ri]:
                    vector.wait_ge(SEM_SQ, gsq[('s', r)] + 1)
                    i = sdma_idx[r]
                    vector.reduce_sum(
                        out=st_red[:, r:r + n],
                        in_=ssq[:, i % SQ_SLOTS, :n, :],
                        axis=mybir.AxisListType.X,
                    ).then_inc(SEM_RED, 1)
                cum_s += len(region_s[ri])
                ar, an = region_a[ri]
                ai = adma_idx[ar]
                # a-reduce once per a-chunk (regions and a-chunks are 1:1)
                vector.wait_ge(SEM_F, ai + 1)
                vector.reduce_sum(
                    out=ac_red[:, ar:ar + an],
                    in_=fb2[:, ai % FB_SLOTS, :an, :],
                    axis=mybir.AxisListType.X,
                ).then_inc(SEM_ARED, 1)
                # same-engine RAW: sem incs fire after write-ack (T10), so
                # these waits guarantee st_red/ac_red are fully landed
                vector.wait_ge(SEM_RED, cum_s)
                vector.wait_ge(SEM_ARED, ri + 1)
                vector.tensor_add(out_t[:, r0:r1], st_red[:, r0:r1],
                                  ac_red[:, r0:r1]).then_inc(SEM_ADD, 1)
                adds_done += 1

    nc.compile()
    return nc


def _get_program(weighted: bool):
    if weighted not in _cache:
        if weighted:
            _cache[weighted] = _build(weighted)
        else:
            _cache[weighted] = _build_manual()
    return _cache[weighted]


def _run(states2d, actions2d, q, r, weighted, trace=False):
    from concourse.bass_utils import run_bass_kernel_spmd

    nc = _get_program(weighted)
    in_maps = []
    for c in range(NCORES):
        m = {
            "states": states2d[c * RPC:(c + 1) * RPC],
            "actions": actions2d[c * RPC:(c + 1) * RPC],
        }
        if weighted:
            m["qlog"] = q
            m["rlog"] = r
        in_maps.append(m)
    res = run_bass_kernel_spmd(nc, in_maps, list(range(NCORES)), trace=trace)
    out = np.concatenate([np.asarray(res.results[c]["cost"]) for c in range(NCORES)])
    return out.astype(np.float32, copy=False), res


def kernel(states, actions, q_diag_log, r_diag_log):
    states2d = np.ascontiguousarray(np.asarray(states, dtype=np.float32)).reshape(BT, DS)
    actions2d = np.ascontiguousarray(np.asarray(actions, dtype=np.float32)).reshape(BT, DA)
    q = np.ascontiguousarray(np.asarray(q_diag_log, dtype=np.float32))
    r = np.ascontiguousarray(np.asarray(r_diag_log, dtype=np.float32))
    weighted = bool(np.any(q != 0.0) or np.any(r != 0.0))
    out, _ = _run(states2d, actions2d, q, r, weighted)
    return out


# revision 17
# speedup vs baseline: 1.1355x; 1.1355x over previous
"""Trainium2 Bass kernel for nn_CostLearning quadratic cost:

    cost[i] = sum_d exp(q_diag_log[d]) * states[i,d]^2
            + sum_d exp(r_diag_log[d]) * actions[i,d]^2

Sharding: pure data parallel over B*T rows across 8 NeuronCores.
Per core: rows are laid out so SBUF partition p owns 256 *consecutive*
rows of the core's shard -> every DMA is 128 partitions x large
contiguous runs, and the d-reduction is a free-axis (X) segmented
reduce on the vector engine.

v2 layout (from trace analysis of v1 @ 68.1us):
  exec window = main_start .. last_teardown_event. v1 breakdown:
  2.2us first-DMA latency + 50.2us stream (SDMA ~100% busy @ ~400GB/s)
  + 5.3us compute drain + 1.8us store + 8.6us sem-teardown ladder.
  v2 attacks the non-stream parts:
  - tapered chunks: big (64 rows/partition, 4MB) early for low
    instruction count, tiny (4 rows) at the end so the post-stream
    drain is ~1.5us instead of 7us
  - squares written as bf16 (inputs stay f32): DVE segmented reduce
    runs at 2x, so the last chunk's reduce is short; f32 accumulate
    in the reduce keeps error ~1e-3 << 2e-2 gate
  - output stored in 5 pieces, 4 of them mid-stream; only a 16-row
    (8KB) store remains after the last input chunk
  - fewer total instructions -> shorter end-of-kernel semaphore
    teardown ladder
"""

import numpy as np

B, T, DS, DA = 128, 2048, 128, 32
BT = B * T
NCORES = 8
RPC = BT // NCORES        # rows per core = 32768
P = 128                   # SBUF partitions
NPP = RPC // P            # rows per partition = 256

# states chunks: uniform 1MB (16 rows/partition) for a fine-grained
# DMA/ACT/DVE pipeline, tapered tail (8,4,4) so the post-stream
# square+reduce drain is tiny. Lumpy 4MB chunks measurably stall the
# tail (v3: big reduces serialize ahead of the small tail chunks).
S_SCHED = [16] * 15 + [8, 4, 4]
assert sum(S_SCHED) == NPP

_cache = {}


def _build(weighted: bool):
    import concourse.bacc as bacc
    import concourse.bass as bass
    import concourse.tile as tile
    from concourse import mybir

    f32 = mybir.dt.float32
    bf16 = mybir.dt.bfloat16
    # all-bf16 intermediates: DVE's 2x_1P perf mode (2 elem/cycle)
    # requires every src AND dst of tensor_reduce to be 2-byte. The
    # reduce ALU accumulates in f32 internally; only the final write
    # rounds to bf16, so the error is ~2^-9 per cost term, well under
    # the 2e-2 gate.
    sq_dt = f32 if weighted else bf16
    red_dt = f32 if weighted else bf16
    nc = bacc.Bacc("TRN2", target_bir_lowering=False, debug=False)

    states = nc.dram_tensor("states", [RPC, DS], f32, kind="ExternalInput")
    actions = nc.dram_tensor("actions", [RPC, DA], f32, kind="ExternalInput")
    if weighted:
        qlog = nc.dram_tensor("qlog", [DS], f32, kind="ExternalInput")
        rlog = nc.dram_tensor("rlog", [DA], f32, kind="ExternalInput")
    cost = nc.dram_tensor("cost", [RPC], f32, kind="ExternalOutput")

    # partition p owns shard rows [p*NPP, (p+1)*NPP)
    sview = states[:].rearrange("(p n) d -> p n d", p=P)    # [128, 256, 128]
    aview = actions[:].rearrange("(p n) d -> p n d", p=P)   # [128, 256, 32]
    oview = cost[:].rearrange("(p n) -> p n", p=P)          # [128, 256]

    s_max = max(S_SCHED)
    a_max = 64

    with tile.TileContext(nc) as tc:
        with (
            tc.tile_pool(name="sio", bufs=6) as sio,
            tc.tile_pool(name="ssqp", bufs=4) as ssqp,
            tc.tile_pool(name="aio", bufs=3) as aio,
            tc.tile_pool(name="asqp", bufs=3) as asqp,
            tc.tile_pool(name="accp", bufs=1) as accp,
        ):
            st_red = accp.tile([P, NPP], red_dt)
            ac_red = accp.tile([P, NPP], red_dt)
            out_t = accp.tile([P, NPP], f32)

            if weighted:
                # exp(weights), broadcast to all partitions and tiled
                # along the free axis to match one chunk's [P, n, d]
                qrep = accp.tile([P, s_max, DS], f32)
                rrep = accp.tile([P, a_max, DA], f32)
                qap = qlog[:]
                rap = rlog[:]
                qb = bass.AP(tensor=qap.tensor, offset=qap.offset,
                             ap=[[0, P], [0, s_max], [1, DS]])
                rb = bass.AP(tensor=rap.tensor, offset=rap.offset,
                             ap=[[0, P], [0, a_max], [1, DA]])
                nc.gpsimd.dma_start(out=qrep, in_=qb)
                nc.gpsimd.dma_start(out=rrep, in_=rb)
                nc.scalar.activation(qrep, qrep,
                                     mybir.ActivationFunctionType.Exp)
                nc.scalar.activation(rrep, rrep,
                                     mybir.ActivationFunctionType.Exp)

            def do_schunk(row0, n):
                s_t = sio.tile([P, s_max, DS], f32, name="s_t")
                nc.sync.dma_start(out=s_t[:, :n, :],
                                  in_=sview[:, row0:row0 + n, :])
                ssq = ssqp.tile([P, s_max, DS], sq_dt, name="ssq")
                nc.scalar.activation(ssq[:, :n, :], s_t[:, :n, :],
                                     mybir.ActivationFunctionType.Square)
                if weighted:
                    nc.vector.tensor_mul(ssq[:, :n, :], ssq[:, :n, :],
                                         qrep[:, :n, :])
                with nc.allow_low_precision("bf16 cost partials; gate is 2e-2"):
                    nc.vector.reduce_sum(
                        out=st_red[:, row0:row0 + n],
                        in_=ssq[:, :n, :],
                        axis=mybir.AxisListType.X,
                    )

            def do_achunk(row0, n):
                a_t = aio.tile([P, a_max, DA], f32, name="a_t")
                nc.sync.dma_start(out=a_t[:, :n, :],
                                  in_=aview[:, row0:row0 + n, :])
                asq = asqp.tile([P, a_max, DA], sq_dt, name="asq")
                nc.scalar.activation(asq[:, :n, :], a_t[:, :n, :],
                                     mybir.ActivationFunctionType.Square)
                if weighted:
                    nc.vector.tensor_mul(asq[:, :n, :], asq[:, :n, :],
                                         rrep[:, :n, :])
                with nc.allow_low_precision("bf16 cost partials; gate is 2e-2"):
                    nc.vector.reduce_sum(
                        out=ac_red[:, row0:row0 + n],
                        in_=asq[:, :n, :],
                        axis=mybir.AxisListType.X,
                    )

            def finalize(r0, r1, store0=None, last=False):
                # add this region; store is a (row0, row1) range that may
                # cover several finalized regions. Mid-stream stores go on
                # the idle gpsimd (SWDGE) queue: HWDGE rings drain FIFO
                # per issuing engine, so a compute-gated store on the sync
                # ring would stall every later input DMA behind it. The
                # final store uses sync (lower latency; ring is empty by
                # then).
                nc.vector.tensor_add(out_t[:, r0:r1], st_red[:, r0:r1],
                                     ac_red[:, r0:r1])
                if store0 is not None:
                    eng = nc.sync if last else nc.gpsimd
                    eng.dma_start(out=oview[:, store0:r1],
                                  in_=out_t[:, store0:r1])

            # explicit interleaved emission: 1MB states chunks drive the
            # stream; action chunks and finalize/stores slot in as their
            # row ranges complete.
            do_schunk(0, 16); do_schunk(16, 16)
            do_achunk(0, 64)
            do_schunk(32, 16); do_schunk(48, 16)
            do_schunk(64, 16); do_schunk(80, 16)
            finalize(0, 64, store0=0)
            do_achunk(64, 64)
            do_schunk(96, 16); do_schunk(112, 16)
            do_schunk(128, 16); do_schunk(144, 16)
            finalize(64, 128, store0=64)
            do_achunk(128, 64)
            do_schunk(160, 16); do_schunk(176, 16)
            do_schunk(192, 16); do_schunk(208, 16)
            finalize(128, 192, store0=128)
            do_achunk(192, 32)
            do_schunk(224, 16)
            do_achunk(224, 16)
            finalize(192, 240, store0=192)
            do_schunk(240, 8)
            do_achunk(240, 16)
            do_schunk(248, 4)
            do_schunk(252, 4)
            finalize(240, 256, store0=240, last=True)

    nc.compile()
    return nc


# (kind, row0, nrows) in DMA issue order; states drive the stream
DMA_ORDER = [
    ('s', 0, 16), ('s', 16, 16), ('a', 0, 64), ('s', 32, 16), ('s', 48, 16),
    ('s', 64, 16), ('s', 80, 16), ('a', 64, 64), ('s', 96, 16), ('s', 112, 16),
    ('s', 128, 16), ('s', 144, 16), ('a', 128, 64), ('s', 160, 16), ('s', 176, 16),
    ('s', 192, 16), ('s', 208, 16), ('a', 192, 32), ('s', 224, 8), ('s', 232, 8),
    ('a', 224, 16), ('s', 240, 4), ('s', 244, 4), ('a', 240, 16), ('s', 248, 4),
    ('s', 252, 4),
]
assert sum(n for kk, _, n in DMA_ORDER if kk == 's') == NPP
assert sum(n for kk, _, n in DMA_ORDER if kk == 'a') == NPP

REGIONS = [(0, 64), (64, 128), (128, 192), (192, 224), (224, 240), (240, 256)]

S_SLOTS, SQ_SLOTS = 7, 6
A_SLOTS, ASQ_SLOTS, FB_SLOTS = 3, 3, 3
SD_LANES, AD_LANES = 8, 4      # must exceed in-flight DMA count per kind
S_MAX, A_MAX = 16, 64


def _build_manual():
    """Raw-bass manual pipeline (unweighted path).

    Engine roles (per-partition busy-time budget vs ~50us DMA stream):
      sync   : input DMAs in stream order (HWDGE ring stays pure) +
               final store
      scalar : states squares only (~38us)
      gpsimd : action squares (tensor_mul) + action fold tree 32->16->8
               + mid-stream stores + end-of-kernel sem reset (~28us)
      vector : segmented reduces + region adds (~43us)

    Synchronization rules learned on HW (all enforced below):
      - a single DMA completion sem aliases between overlapping DMAs
        (+16 incs arrive per-SDMA-engine, unordered across DMAs); lanes
        must exceed the in-flight window per kind
      - same-engine back-to-back ops do NOT order write-tails before the
        next op's reads (prefetch starts at T8, writes ack at T10) -- any
        same-engine RAW needs a sem wait (incs fire at T10)
      - the NTFF profiling path re-executes the NEFF, so all sems must
        be reset to 0 at the end (gpsimd dma_reset + sem_clear after the
        final store completions)
    """
    import concourse.bacc as bacc
    from concourse import mybir
    from contextlib import ExitStack

    f32 = mybir.dt.float32
    nc = bacc.Bacc("TRN2", target_bir_lowering=False, debug=False)

    states = nc.dram_tensor("states", [RPC, DS], f32, kind="ExternalInput")
    actions = nc.dram_tensor("actions", [RPC, DA], f32, kind="ExternalInput")
    cost = nc.dram_tensor("cost", [RPC], f32, kind="ExternalOutput")

    sview = states[:].rearrange("(p n) d -> p n d", p=P)    # [128, 256, 128]
    aview = actions[:].rearrange("(p n) d -> p n d", p=P)   # [128, 256, 32]
    oview = cost[:].rearrange("(p n) -> p n", p=P)          # [128, 256]

    # --- static schedule bookkeeping ---------------------------------
    s_chunks = [(r, n) for kk, r, n in DMA_ORDER if kk == 's']
    a_chunks = [(r, n) for kk, r, n in DMA_ORDER if kk == 'a']
    sdma_idx = {r: i for i, (r, n) in enumerate(s_chunks)}
    adma_idx = {r: i for i, (r, n) in enumerate(a_chunks)}
    region_s = {i: [] for i in range(len(REGIONS))}
    region_a = {}
    for i, (r0, r1) in enumerate(REGIONS):
        for (r, n) in s_chunks:
            if r0 <= r < r1:
                region_s[i].append((r, n))
        for (r, n) in a_chunks:
            if r0 <= r < r1 or (r < r0 and r + n >= r1):
                region_a.setdefault(i, (r, n))
    # DVE s-reduce completion order (region-major == s_chunks order here)
    dve_s_order = []
    for i in range(len(REGIONS)):
        dve_s_order.extend(region_s[i])
    s_red_pos = {r: i for i, (r, n) in enumerate(dve_s_order)}

    es = ExitStack()
    with es:
        s_t = es.enter_context(nc.sbuf_tensor("s_t", [P, S_SLOTS, S_MAX, DS], f32))
        ssq = es.enter_context(nc.sbuf_tensor("ssq", [P, SQ_SLOTS, S_MAX, DS], f32))
        a_t = es.enter_context(nc.sbuf_tensor("a_t", [P, A_SLOTS, A_MAX, DA], f32))
        asq = es.enter_context(nc.sbuf_tensor("asq", [P, ASQ_SLOTS, A_MAX, DA], f32))
        fb1 = es.enter_context(nc.sbuf_tensor("fb1", [P, FB_SLOTS, A_MAX, 16], f32))
        fb2 = es.enter_context(nc.sbuf_tensor("fb2", [P, FB_SLOTS, A_MAX, 8], f32))
        st_red = es.enter_context(nc.sbuf_tensor("st_red", [P, NPP], f32))
        ac_red = es.enter_context(nc.sbuf_tensor("ac_red", [P, NPP], f32))
        out_t = es.enter_context(nc.sbuf_tensor("out_t", [P, NPP], f32))
        zbias = es.enter_context(nc.sbuf_tensor("zbias", [P, 1], f32))
        SEM_Z = es.enter_context(nc.semaphore("SEM_Z"))
        SD = [es.enter_context(nc.semaphore(f"SEM_SD{j}")) for j in range(SD_LANES)]
        AD = [es.enter_context(nc.semaphore(f"SEM_AD{j}")) for j in range(AD_LANES)]
        SEM_SQ = es.enter_context(nc.semaphore("SEM_SQ"))      # ACT s-squares
        SEM_ASQ = es.enter_context(nc.semaphore("SEM_ASQ"))    # gpsimd a-squares
        SEM_F = es.enter_context(nc.semaphore("SEM_F"))        # gpsimd fold2
        SEM_F1 = es.enter_context(nc.semaphore("SEM_F1"))      # gpsimd fold1
        SEM_RED = es.enter_context(nc.semaphore("SEM_RED"))    # DVE s-reduces
        SEM_ARED = es.enter_context(nc.semaphore("SEM_ARED"))  # DVE a-reduces
        SEM_ADD = es.enter_context(nc.semaphore("SEM_ADD"))    # DVE region adds
        SEM_ST = es.enter_context(nc.semaphore("SEM_ST"))      # store DMAs
        all_sems = [SEM_Z, *SD, *AD, SEM_SQ, SEM_ASQ, SEM_F, SEM_F1,
                    SEM_RED, SEM_ARED, SEM_ADD, SEM_ST]
        sem_nums = sorted(s.num for s in all_sems)
        assert sem_nums == list(range(sem_nums[0], sem_nums[-1] + 1))
        sem_range = range(sem_nums[0], sem_nums[-1] + 1)
        block = es.enter_context(nc.Block())

        @block.sync
        def _(sync):
            for kk, r, n in DMA_ORDER:
                if kk == 's':
                    i = sdma_idx[r]
                    if i >= S_SLOTS:
                        # slot reuse: ACT squared chunk i-S_SLOTS (s-sq #i-S_SLOTS+1)
                        sync.wait_ge(SEM_SQ, i - S_SLOTS + 1)
                    sync.dma_start(
                        out=s_t[:, i % S_SLOTS, :n, :], in_=sview[:, r:r + n, :]
                    ).then_inc(SD[i % SD_LANES], 16)
                else:
                    i = adma_idx[r]
                    if i >= A_SLOTS:
                        # slot reuse: gpsimd squared chunk i-A_SLOTS
                        sync.wait_ge(SEM_ASQ, i - A_SLOTS + 1)
                    sync.dma_start(
                        out=a_t[:, i % A_SLOTS, :n, :], in_=aview[:, r:r + n, :]
                    ).then_inc(AD[i % AD_LANES], 16)
            # final store [192:256] after the last region add
            sync.wait_ge(SEM_ADD, len(REGIONS))
            sync.dma_start(out=oview[:, 192:], in_=out_t[:, 192:]).then_inc(
                SEM_ST, 16)

        @block.scalar
        def _(scalar):
            scalar.wait_ge(SEM_Z, 1)
            for i, (r, n) in enumerate(s_chunks):
                scalar.wait_ge(SD[i % SD_LANES], 16 * (i // SD_LANES + 1))
                if i >= SQ_SLOTS:
                    # ssq slot reuse: DVE reduce of chunk i-SQ_SLOTS done
                    pr, _ = s_chunks[i - SQ_SLOTS]
                    scalar.wait_ge(SEM_RED, s_red_pos[pr] + 1)
                scalar.activation(
                    ssq[:, i % SQ_SLOTS, :n, :], s_t[:, i % S_SLOTS, :n, :],
                    mybir.ActivationFunctionType.Square,
                    bias=zbias[:, :1],
                ).then_inc(SEM_SQ, 1)

        @block.gpsimd
        def _(gpsimd):
            # raw bass has no const pool; ACT Square needs a bias AP
            gpsimd.memset(zbias[:, :], 0.0).then_inc(SEM_Z, 1)
            for i, (r, n) in enumerate(a_chunks):
                sl = i % ASQ_SLOTS
                fsl = i % FB_SLOTS
                gpsimd.wait_ge(AD[i % AD_LANES], 16 * (i // AD_LANES + 1))
                # a-square on gpsimd (asq slot reuse is same-engine WAR
                # vs fold1(i-ASQ_SLOTS): its reads are long done)
                gpsimd.tensor_mul(asq[:, sl, :n, :], a_t[:, sl_in(i), :n, :],
                                  a_t[:, sl_in(i), :n, :]).then_inc(SEM_ASQ, 1)
                # same-engine RAW chains need sem waits (write-ack ordering)
                gpsimd.wait_ge(SEM_ASQ, i + 1)
                gpsimd.tensor_add(fb1[:, fsl, :n, :],
                                  asq[:, sl, :n, 0:16], asq[:, sl, :n, 16:32]
                                  ).then_inc(SEM_F1, 1)
                if i >= FB_SLOTS:
                    # fb2 slot reuse: DVE a-reduce of chunk i-FB_SLOTS done
                    gpsimd.wait_ge(SEM_ARED, i - FB_SLOTS + 1)
                gpsimd.wait_ge(SEM_F1, i + 1)
                gpsimd.tensor_add(fb2[:, fsl, :n, :],
                                  fb1[:, fsl, :n, 0:8], fb1[:, fsl, :n, 8:16]
                                  ).then_inc(SEM_F, 1)
                # mid-stream stores for regions 0..2 ride the SWDGE queue:
                # a compute-gated store on the sync HWDGE ring would stall
                # every later input DMA behind it
                if i in (1, 2, 3):
                    reg = i - 1
                    gpsimd.wait_ge(SEM_ADD, reg + 1)
                    r0, r1 = REGIONS[reg]
                    gpsimd.dma_start(out=oview[:, r0:r1],
                                     in_=out_t[:, r0:r1]).then_inc(SEM_ST, 16)
            # completion + reset for re-execution (profiling re-runs the
            # NEFF): every sem's final inc is upstream of the 4 store
            # completions, so clearing here is race-free
            gpsimd.wait_ge(SEM_ST, 64)
            gpsimd.dma_reset(sem_range)
            gpsimd.sem_clear(sem_range)

        @block.vector
        def _(vector):
            cum_s = 0
            for ri, (r0, r1) in enumerate(REGIONS):
                for (r, n) in region_s[ri]:
                    i = sdma_idx[r]
                    vector.wait_ge(SEM_SQ, i + 1)
                    vector.reduce_sum(
                        out=st_red[:, r:r + n],
                        in_=ssq[:, i % SQ_SLOTS, :n, :],
                        axis=mybir.AxisListType.X,
                    ).then_inc(SEM_RED, 1)
                cum_s += len(region_s[ri])
                ar, an = region_a[ri]
                ai = adma_idx[ar]
                vector.wait_ge(SEM_F, ai + 1)
                vector.reduce_sum(
                    out=ac_red[:, ar:ar + an],
                    in_=fb2[:, ai % FB_SLOTS, :an, :],
                    axis=mybir.AxisListType.X,
                ).then_inc(SEM_ARED, 1)
                # same-engine RAW: sem incs fire after write-ack, so these
                # waits guarantee st_red/ac_red fully landed before the add
                vector.wait_ge(SEM_RED, cum_s)
                vector.wait_ge(SEM_ARED, ri + 1)
                vector.tensor_add(out_t[:, r0:r1], st_red[:, r0:r1],
                                  ac_red[:, r0:r1]).then_inc(SEM_ADD, 1)

    nc.compile()
    return nc


def sl_in(i):
    return i % A_SLOTS


def _get_program(weighted: bool):
    if weighted not in _cache:
        if weighted:
            _cache[weighted] = _build(weighted)
        else:
            _cache[weighted] = _build_manual()
    return _cache[weighted]


def _run(states2d, actions2d, q, r, weighted, trace=False):
    from concourse.bass_utils import run_bass_kernel_spmd

    nc = _get_program(weighted)
    in_maps = []
    for c in range(NCORES):
        m = {
            "states": states2d[c * RPC:(c + 1) * RPC],
            "actions": actions2d[c * RPC:(c + 1) * RPC],
        }
        if weighted:
            m["qlog"] = q
            m["rlog"] = r
        in_maps.append(m)
    res = run_bass_kernel_spmd(nc, in_maps, list(range(NCORES)), trace=trace)
    out = np.concatenate([np.asarray(res.results[c]["cost"]) for c in range(NCORES)])
    return out.astype(np.float32, copy=False), res


def kernel(states, actions, q_diag_log, r_diag_log):
    states2d = np.ascontiguousarray(np.asarray(states, dtype=np.float32)).reshape(BT, DS)
    actions2d = np.ascontiguousarray(np.asarray(actions, dtype=np.float32)).reshape(BT, DA)
    q = np.ascontiguousarray(np.asarray(q_diag_log, dtype=np.float32))
    r = np.ascontiguousarray(np.asarray(r_diag_log, dtype=np.float32))
    weighted = bool(np.any(q != 0.0) or np.any(r != 0.0))
    out, _ = _run(states2d, actions2d, q, r, weighted)
    return out
